# revision 28
# baseline (speedup 1.0000x reference)
"""Trainium2 Bass kernel for nn_Graph_Layer_44787918963014 (gnn_message_passing).

out = ALPHA * softmax(q k^T) @ x @ weight + (1-ALPHA) * G_time @ x @ weight_time
with q = x@W0.T, k = x@W1.T, G_time the normalized (n-|i-j|) Toeplitz affinity.

Strategy (8 NeuronCores, row-sharded: core c owns query rows [c*1024, (c+1)*1024)):
  host prep : q/k projections (fp32 GEMMs); G_time path computed exactly via
              prefix sums (Toeplitz closed form) -> out_time, no device work.
  device    : per j-block of 128 keys and m-half of 512 queries:
              S^T = k_j^T q_m  (single float32r matmul, ~fp32-accurate),
              E^T = exp(S^T - 75) via ACT (constant bias cancels in the
              normalization, so no per-row max pass is needed),
              U^T[d,m] += x_j^T E_j accumulated fully in PSUM across all 64
              j-blocks (4 banks), Z partials accumulated on DVE.
  host epi  : Z = sum(Zpart); out = (U^T)^T @ weight * (ALPHA/Z) + out_time.

Self-contained: shapes hardcoded, no sibling imports. Falls back to an exact
host computation if the device path fails for any reason.
"""
import sys
import traceback
import numpy as np

N, IN, FEAT, NOUT = 8192, 512, 128, 512
ALPHA = 0.5
NCORES = 8
NLOC = N // NCORES     # 1024 query rows per core
P = 128                # j-block (keys per block) and PE partition width
NBLK = N // P          # 64 j-blocks
HCOLS = 512            # m-half width (one PSUM bank of fp32)
EXP_BIAS = -75.0       # constant exp offset; cancels exactly in softmax


def _host_reference(x, W0, W1, weight, weight_time):
    x = np.asarray(x, np.float32)
    q = x @ np.asarray(W0, np.float32).T
    k = x @ np.asarray(W1, np.float32).T
    s = q @ k.T
    s -= s.max(1, keepdims=True)
    e = np.exp(s, dtype=np.float32)
    g = e / e.sum(1, keepdims=True)
    i = np.arange(N, dtype=np.float32)
    M = (N - np.abs(i[:, None] - i[None, :]))
    M /= M.sum(1, keepdims=True)
    out = ALPHA * (g @ x) @ np.asarray(weight, np.float32)
    out += (1.0 - ALPHA) * (M @ x) @ np.asarray(weight_time, np.float32)
    return out.astype(np.float32)


def _toeplitz_out_time(x, weight_time):
    """(1-ALPHA) * (G_time @ x) @ weight_time via the Toeplitz closed form.

    M[i,j] = N - |i-j|;  (M@x)[i] = N*T0 - (2i*P0[i] - 2*P1[i] + T1 - i*T0)
    with P0/P1 prefix sums of x and j*x (fp64 for the cancellation-heavy sums).
    """
    i = np.arange(N, dtype=np.float64)[:, None]
    x64 = x.astype(np.float64)
    P0 = np.cumsum(x64, 0)
    P1 = np.cumsum(i * x64, 0)
    T0, T1 = P0[-1][None, :], P1[-1][None, :]
    Srow = (N * N - (i * (i + 1) / 2 + (N - 1 - i) * (N - i) / 2))
    Mx = (N * T0 - (2 * i * P0 - 2 * P1 + T1 - i * T0)) / Srow
    return ((1.0 - ALPHA) * (Mx.astype(np.float32) @ weight_time)).astype(np.float32)


def _build_nc():
    from concourse import bacc, tile, mybir
    from contextlib import ExitStack
    F32 = mybir.dt.float32
    F32R = mybir.dt.float32r
    BF16 = mybir.dt.bfloat16

    nc = bacc.Bacc()
    kt_d = nc.declare_dram_parameter("kt", [FEAT, N], F32R, isOutput=False)
    qt_d = nc.declare_dram_parameter("qt", [FEAT, NLOC], F32R, isOutput=False)
    xb_d = nc.declare_dram_parameter("xb", [N, IN], BF16, isOutput=False)
    o_ut = nc.declare_dram_parameter("o_ut", [IN, NLOC], F32, isOutput=True)
    o_z = nc.declare_dram_parameter("o_z", [P, NLOC], F32, isOutput=True)

    # k chunk column boundaries: fine 256-col chunks (2 key blocks each).
    # Tile deps are whole-tile, so each chunk is its own tile and S_b waits
    # only on the chunk containing its key block. Fine chunks let the DMA
    # schedule interleave k and x at per-block grain (queue rate ~210 GB/s
    # vs ~179 GB/s steady demand, so supply must never bunch up).
    KB = list(range(0, N + 1, 256))

    with tile.TileContext(nc) as tc, ExitStack() as ctx:
        cst = ctx.enter_context(tc.tile_pool(name="cst", bufs=1))
        xpool = ctx.enter_context(tc.tile_pool(name="xp", bufs=1))
        epool = ctx.enter_context(tc.tile_pool(name="ep", bufs=4))
        stg = ctx.enter_context(tc.tile_pool(name="stg", bufs=4))
        pss = ctx.enter_context(tc.tile_pool(name="pss", bufs=4, space="PSUM"))
        psu = ctx.enter_context(tc.tile_pool(name="psu", bufs=1, space="PSUM"))

        # separate q tile per m-half (whole-tile deps again)
        qtiles = [cst.tile([FEAT, HCOLS], F32R, name=f"q{h}")
                  for h in range(NLOC // HCOLS)]
        kchunks = [cst.tile([FEAT, KB[c + 1] - KB[c]], F32R, name=f"kc{c}")
                   for c in range(len(KB) - 1)]
        xtiles = [xpool.tile([P, IN], BF16, name=f"x{b}")
                  for b in range(NBLK)]

        def dma_k(c):
            nc.sync.dma_start(kchunks[c][:], kt_d[:, KB[c]:KB[c + 1]])

        def dma_x(b0, b1):
            for b in range(b0, b1):
                nc.sync.dma_start(xtiles[b][:], xb_d[b * P:(b + 1) * P, :])

        # supply-ordered DMA schedule: q first, then k chunks woven between
        # x blocks (1 chunk : 2 blocks cadence, chunks running ~2 ahead) so
        # both S_b (prefetched ahead) and U_b stay fed at ~210 GB/s
        nc.sync.dma_start(qtiles[0][:], qt_d[:, 0:HCOLS])
        dma_k(0)
        dma_x(0, 1)
        dma_k(1)
        dma_x(1, 2)
        for c in range(2, len(KB) - 1):
            dma_k(c)
            dma_x(2 * c - 2, 2 * c)
        dma_x(2 * (len(KB) - 1) - 2, NBLK)
        nc.sync.dma_start(qtiles[1][:], qt_d[:, HCOLS:NLOC])

        bias = cst.tile([P, 1], F32, name="bias")
        nc.vector.memset(bias[:], EXP_BIAS)
        zacc = cst.tile([P, NLOC], F32, name="zacc")
        nc.vector.memset(zacc[:], 0.0)

        # PE warm-up: ramp the tensor-engine clock while DMAs land
        wl = cst.tile([P, 64], BF16, name="wl")
        wr = cst.tile([P, P], BF16, name="wr")
        nc.vector.memset(wl[:], 0.0)
        nc.vector.memset(wr[:], 0.0)
        utiles = [psu.tile([P, HCOLS], F32, name=f"u{d}") for d in range(4)]
        for _ in range(8):
            nc.tensor.matmul(utiles[0][0:64, 0:P], wl[:], wr[:],
                             start=True, stop=True)

        for h in range(NLOC // HCOLS):
            hs = slice(h * HCOLS, (h + 1) * HCOLS)
            stash = {}

            def do_scores(b):
                sp = pss.tile([P, HCOLS], F32, tag="s")
                c, off = b // 2, (b % 2) * P
                nc.tensor.matmul(sp[:], kchunks[c][:, off:off + P],
                                 qtiles[h][:], start=True, stop=True)
                stash[b] = sp

            do_scores(0)
            do_scores(1)
            do_scores(2)
            for b in range(NBLK):
                if b + 3 < NBLK:
                    do_scores(b + 3)
                et = epool.tile([P, HCOLS], BF16, tag="e")
                nc.scalar.activation(et[:], stash.pop(b)[:],
                                     mybir.ActivationFunctionType.Exp,
                                     bias=bias[:])
                for d in range(4):
                    nc.tensor.matmul(utiles[d][:],
                                     xtiles[b][:, d * P:(d + 1) * P], et[:],
                                     start=(b == 0), stop=(b == NBLK - 1))
                nc.vector.tensor_tensor(zacc[:, hs], zacc[:, hs], et[:],
                                        mybir.AluOpType.add)

            nc.sync.dma_start(o_z[:, hs], zacc[:, hs])
            for d in range(4):
                st = stg.tile([P, HCOLS], F32, tag="st")
                nc.vector.tensor_copy(st[:], utiles[d][:])
                nc.sync.dma_start(o_ut[d * P:(d + 1) * P, hs], st[:])

    if not nc.is_finalized():
        nc.finalize()
    return nc


def _device_kernel(x, W0, W1, weight, weight_time, trace=False):
    sys.path.insert(0, "/opt/trn_rl_repo")
    import ml_dtypes
    from concourse.bass_utils import run_bass_kernel_spmd

    bf = ml_dtypes.bfloat16
    x = np.asarray(x, np.float32)
    W0 = np.asarray(W0, np.float32)
    W1 = np.asarray(W1, np.float32)
    weight = np.asarray(weight, np.float32)
    weight_time = np.asarray(weight_time, np.float32)

    q = x @ W0.T                       # [N, FEAT] fp32
    k = x @ W1.T
    kT = np.ascontiguousarray(k.T)     # [FEAT, N]
    qT = np.ascontiguousarray(q.T)
    xb = x.astype(bf)
    out_time = _toeplitz_out_time(x, weight_time)

    nc = _build_nc()
    in_maps = [dict(kt=kT, qt=np.ascontiguousarray(qT[:, c * NLOC:(c + 1) * NLOC]),
                    xb=xb) for c in range(NCORES)]

    kwargs = {}
    if trace:
        kwargs = dict(trace=True, trace_cores=list(range(NCORES)))
    res = run_bass_kernel_spmd(nc, in_maps, list(range(NCORES)), **kwargs)

    out = np.empty((N, NOUT), np.float32)
    for c in range(NCORES):
        r = res.results[c]
        sl = slice(c * NLOC, (c + 1) * NLOC)
        Z = r["o_z"].sum(0, dtype=np.float64).astype(np.float32)   # [NLOC]
        attn = (r["o_ut"].T @ weight) * (ALPHA / Z)[:, None]
        out[sl] = attn + out_time[sl]
    return out, res


def kernel(**inputs):
    try:
        out, _ = _device_kernel(**inputs)
        ref_dtype = np.asarray(inputs["x"]).dtype
        return out.astype(ref_dtype)
    except Exception:
        traceback.print_exc()
        sys.stderr.write("device path failed; using host fallback\n")
        return _host_reference(**inputs)


# revision 29
# speedup vs baseline: 1.0075x; 1.0075x over previous
"""Trainium2 Bass kernel for nn_Graph_Layer_44787918963014 (gnn_message_passing).

out = ALPHA * softmax(q k^T) @ x @ weight + (1-ALPHA) * G_time @ x @ weight_time
with q = x@W0.T, k = x@W1.T, G_time the normalized (n-|i-j|) Toeplitz affinity.

Strategy (8 NeuronCores, row-sharded: core c owns query rows [c*1024, (c+1)*1024)):
  host prep : q/k projections (fp32 GEMMs); G_time path computed exactly via
              prefix sums (Toeplitz closed form) -> out_time, no device work.
  device    : per j-block of 128 keys and m-half of 512 queries:
              S^T = k_j^T q_m  (single float32r matmul, ~fp32-accurate),
              E^T = exp(S^T - 75) via ACT (constant bias cancels in the
              normalization, so no per-row max pass is needed),
              U^T[d,m] += x_j^T E_j accumulated fully in PSUM across all 64
              j-blocks (4 banks), Z partials accumulated on DVE.
  host epi  : Z = sum(Zpart); out = (U^T)^T @ weight * (ALPHA/Z) + out_time.

Self-contained: shapes hardcoded, no sibling imports. Falls back to an exact
host computation if the device path fails for any reason.
"""
import sys
import traceback
import numpy as np

N, IN, FEAT, NOUT = 8192, 512, 128, 512
ALPHA = 0.5
NCORES = 8
NLOC = N // NCORES     # 1024 query rows per core
P = 128                # j-block (keys per block) and PE partition width
NBLK = N // P          # 64 j-blocks
HCOLS = 512            # m-half width (one PSUM bank of fp32)
EXP_BIAS = -75.0       # constant exp offset; cancels exactly in softmax


def _host_reference(x, W0, W1, weight, weight_time):
    x = np.asarray(x, np.float32)
    q = x @ np.asarray(W0, np.float32).T
    k = x @ np.asarray(W1, np.float32).T
    s = q @ k.T
    s -= s.max(1, keepdims=True)
    e = np.exp(s, dtype=np.float32)
    g = e / e.sum(1, keepdims=True)
    i = np.arange(N, dtype=np.float32)
    M = (N - np.abs(i[:, None] - i[None, :]))
    M /= M.sum(1, keepdims=True)
    out = ALPHA * (g @ x) @ np.asarray(weight, np.float32)
    out += (1.0 - ALPHA) * (M @ x) @ np.asarray(weight_time, np.float32)
    return out.astype(np.float32)


def _toeplitz_out_time(x, weight_time):
    """(1-ALPHA) * (G_time @ x) @ weight_time via the Toeplitz closed form.

    M[i,j] = N - |i-j|;  (M@x)[i] = N*T0 - (2i*P0[i] - 2*P1[i] + T1 - i*T0)
    with P0/P1 prefix sums of x and j*x (fp64 for the cancellation-heavy sums).
    """
    i = np.arange(N, dtype=np.float64)[:, None]
    x64 = x.astype(np.float64)
    P0 = np.cumsum(x64, 0)
    P1 = np.cumsum(i * x64, 0)
    T0, T1 = P0[-1][None, :], P1[-1][None, :]
    Srow = (N * N - (i * (i + 1) / 2 + (N - 1 - i) * (N - i) / 2))
    Mx = (N * T0 - (2 * i * P0 - 2 * P1 + T1 - i * T0)) / Srow
    return ((1.0 - ALPHA) * (Mx.astype(np.float32) @ weight_time)).astype(np.float32)


def _build_nc():
    from concourse import bacc, tile, mybir
    from contextlib import ExitStack
    F32 = mybir.dt.float32
    F32R = mybir.dt.float32r
    BF16 = mybir.dt.bfloat16

    nc = bacc.Bacc()
    kt_d = nc.declare_dram_parameter("kt", [FEAT, N], F32R, isOutput=False)
    qt_d = nc.declare_dram_parameter("qt", [FEAT, NLOC], F32R, isOutput=False)
    xb_d = nc.declare_dram_parameter("xb", [N, IN], BF16, isOutput=False)
    o_ut = nc.declare_dram_parameter("o_ut", [IN, NLOC], F32, isOutput=True)
    o_z = nc.declare_dram_parameter("o_z", [P, NLOC], F32, isOutput=True)

    # k chunk column boundaries: fine 256-col chunks (2 key blocks each).
    # Tile deps are whole-tile, so each chunk is its own tile and S_b waits
    # only on the chunk containing its key block. Fine chunks let the DMA
    # schedule interleave k and x at per-block grain (queue rate ~210 GB/s
    # vs ~179 GB/s steady demand, so supply must never bunch up).
    KB = list(range(0, N + 1, 256))

    with tile.TileContext(nc) as tc, ExitStack() as ctx:
        cst = ctx.enter_context(tc.tile_pool(name="cst", bufs=1))
        xpool = ctx.enter_context(tc.tile_pool(name="xp", bufs=1))
        epool = ctx.enter_context(tc.tile_pool(name="ep", bufs=4))
        stg = ctx.enter_context(tc.tile_pool(name="stg", bufs=4))
        pss = ctx.enter_context(tc.tile_pool(name="pss", bufs=4, space="PSUM"))
        psu = ctx.enter_context(tc.tile_pool(name="psu", bufs=1, space="PSUM"))

        # separate q tile per m-half (whole-tile deps again)
        qtiles = [cst.tile([FEAT, HCOLS], F32R, name=f"q{h}")
                  for h in range(NLOC // HCOLS)]
        kchunks = [cst.tile([FEAT, KB[c + 1] - KB[c]], F32R, name=f"kc{c}")
                   for c in range(len(KB) - 1)]
        xtiles = [xpool.tile([P, IN], BF16, name=f"x{b}")
                  for b in range(NBLK)]

        def dma_k(c):
            nc.sync.dma_start(kchunks[c][:], kt_d[:, KB[c]:KB[c + 1]])

        def dma_x(b0, b1):
            for b in range(b0, b1):
                nc.sync.dma_start(xtiles[b][:], xb_d[b * P:(b + 1) * P, :])

        # supply-ordered DMA schedule at the measured ~210 GB/s queue rate:
        # q first, K0+K1 front-loaded (S_0..S_3 are prefetched before U_0
        # in PE order, so blocks 0-3 of k must beat x_0), then 1 k-chunk
        # per 2 x-blocks — k stays ~2 chunks ahead, x lands just-in-time
        nc.sync.dma_start(qtiles[0][:], qt_d[:, 0:HCOLS])
        dma_k(0)
        dma_k(1)
        dma_x(0, 1)
        dma_k(2)
        dma_x(1, 3)
        for c in range(3, len(KB) - 1):
            dma_k(c)
            dma_x(2 * c - 3, 2 * c - 1)
        dma_x(2 * (len(KB) - 1) - 3, NBLK)
        nc.sync.dma_start(qtiles[1][:], qt_d[:, HCOLS:NLOC])

        bias = cst.tile([P, 1], F32, name="bias")
        nc.vector.memset(bias[:], EXP_BIAS)
        zacc = cst.tile([P, NLOC], F32, name="zacc")
        nc.vector.memset(zacc[:], 0.0)

        # PE warm-up: ramp the tensor-engine clock while DMAs land
        wl = cst.tile([P, 64], BF16, name="wl")
        wr = cst.tile([P, P], BF16, name="wr")
        nc.vector.memset(wl[:], 0.0)
        nc.vector.memset(wr[:], 0.0)
        utiles = [psu.tile([P, HCOLS], F32, name=f"u{d}") for d in range(4)]
        for _ in range(8):
            nc.tensor.matmul(utiles[0][0:64, 0:P], wl[:], wr[:],
                             start=True, stop=True)

        for h in range(NLOC // HCOLS):
            hs = slice(h * HCOLS, (h + 1) * HCOLS)
            stash = {}

            def do_scores(b):
                sp = pss.tile([P, HCOLS], F32, tag="s")
                c, off = b // 2, (b % 2) * P
                nc.tensor.matmul(sp[:], kchunks[c][:, off:off + P],
                                 qtiles[h][:], start=True, stop=True)
                stash[b] = sp

            do_scores(0)
            do_scores(1)
            do_scores(2)
            for b in range(NBLK):
                if b + 3 < NBLK:
                    do_scores(b + 3)
                et = epool.tile([P, HCOLS], BF16, tag="e")
                nc.scalar.activation(et[:], stash.pop(b)[:],
                                     mybir.ActivationFunctionType.Exp,
                                     bias=bias[:])
                for d in range(4):
                    nc.tensor.matmul(utiles[d][:],
                                     xtiles[b][:, d * P:(d + 1) * P], et[:],
                                     start=(b == 0), stop=(b == NBLK - 1))
                nc.vector.tensor_tensor(zacc[:, hs], zacc[:, hs], et[:],
                                        mybir.AluOpType.add)

            nc.sync.dma_start(o_z[:, hs], zacc[:, hs])
            for d in range(4):
                st = stg.tile([P, HCOLS], F32, tag="st")
                nc.vector.tensor_copy(st[:], utiles[d][:])
                nc.sync.dma_start(o_ut[d * P:(d + 1) * P, hs], st[:])

    if not nc.is_finalized():
        nc.finalize()
    return nc


def _device_kernel(x, W0, W1, weight, weight_time, trace=False):
    sys.path.insert(0, "/opt/trn_rl_repo")
    import ml_dtypes
    from concourse.bass_utils import run_bass_kernel_spmd

    bf = ml_dtypes.bfloat16
    x = np.asarray(x, np.float32)
    W0 = np.asarray(W0, np.float32)
    W1 = np.asarray(W1, np.float32)
    weight = np.asarray(weight, np.float32)
    weight_time = np.asarray(weight_time, np.float32)

    q = x @ W0.T                       # [N, FEAT] fp32
    k = x @ W1.T
    kT = np.ascontiguousarray(k.T)     # [FEAT, N]
    qT = np.ascontiguousarray(q.T)
    xb = x.astype(bf)
    out_time = _toeplitz_out_time(x, weight_time)

    nc = _build_nc()
    in_maps = [dict(kt=kT, qt=np.ascontiguousarray(qT[:, c * NLOC:(c + 1) * NLOC]),
                    xb=xb) for c in range(NCORES)]

    kwargs = {}
    if trace:
        kwargs = dict(trace=True, trace_cores=list(range(NCORES)))
    res = run_bass_kernel_spmd(nc, in_maps, list(range(NCORES)), **kwargs)

    out = np.empty((N, NOUT), np.float32)
    for c in range(NCORES):
        r = res.results[c]
        sl = slice(c * NLOC, (c + 1) * NLOC)
        Z = r["o_z"].sum(0, dtype=np.float64).astype(np.float32)   # [NLOC]
        attn = (r["o_ut"].T @ weight) * (ALPHA / Z)[:, None]
        out[sl] = attn + out_time[sl]
    return out, res


def kernel(**inputs):
    try:
        out, _ = _device_kernel(**inputs)
        ref_dtype = np.asarray(inputs["x"]).dtype
        return out.astype(ref_dtype)
    except Exception:
        traceback.print_exc()
        sys.stderr.write("device path failed; using host fallback\n")
        return _host_reference(**inputs)


# revision 30
# speedup vs baseline: 1.0084x; 1.0009x over previous
"""Trainium2 Bass kernel for nn_Graph_Layer_44787918963014 (gnn_message_passing).

out = ALPHA * softmax(q k^T) @ x @ weight + (1-ALPHA) * G_time @ x @ weight_time
with q = x@W0.T, k = x@W1.T, G_time the normalized (n-|i-j|) Toeplitz affinity.

Strategy (8 NeuronCores, row-sharded: core c owns query rows [c*1024, (c+1)*1024)):
  host prep : q/k projections (fp32 GEMMs); G_time path computed exactly via
              prefix sums (Toeplitz closed form) -> out_time, no device work.
  device    : per j-block of 128 keys and m-half of 512 queries:
              S^T = k_j^T q_m  (single float32r matmul, ~fp32-accurate),
              E^T = exp(S^T - 75) via ACT (constant bias cancels in the
              normalization, so no per-row max pass is needed),
              U^T[d,m] += x_j^T E_j accumulated fully in PSUM across all 64
              j-blocks (4 banks), Z partials accumulated on DVE.
  host epi  : Z = sum(Zpart); out = (U^T)^T @ weight * (ALPHA/Z) + out_time.

Self-contained: shapes hardcoded, no sibling imports. Falls back to an exact
host computation if the device path fails for any reason.
"""
import sys
import traceback
import numpy as np

N, IN, FEAT, NOUT = 8192, 512, 128, 512
ALPHA = 0.5
NCORES = 8
NLOC = N // NCORES     # 1024 query rows per core
P = 128                # j-block (keys per block) and PE partition width
NBLK = N // P          # 64 j-blocks
HCOLS = 512            # m-half width (one PSUM bank of fp32)
EXP_BIAS = -75.0       # constant exp offset; cancels exactly in softmax


def _host_reference(x, W0, W1, weight, weight_time):
    x = np.asarray(x, np.float32)
    q = x @ np.asarray(W0, np.float32).T
    k = x @ np.asarray(W1, np.float32).T
    s = q @ k.T
    s -= s.max(1, keepdims=True)
    e = np.exp(s, dtype=np.float32)
    g = e / e.sum(1, keepdims=True)
    i = np.arange(N, dtype=np.float32)
    M = (N - np.abs(i[:, None] - i[None, :]))
    M /= M.sum(1, keepdims=True)
    out = ALPHA * (g @ x) @ np.asarray(weight, np.float32)
    out += (1.0 - ALPHA) * (M @ x) @ np.asarray(weight_time, np.float32)
    return out.astype(np.float32)


def _toeplitz_out_time(x, weight_time):
    """(1-ALPHA) * (G_time @ x) @ weight_time via the Toeplitz closed form.

    M[i,j] = N - |i-j|;  (M@x)[i] = N*T0 - (2i*P0[i] - 2*P1[i] + T1 - i*T0)
    with P0/P1 prefix sums of x and j*x (fp64 for the cancellation-heavy sums).
    """
    i = np.arange(N, dtype=np.float64)[:, None]
    x64 = x.astype(np.float64)
    P0 = np.cumsum(x64, 0)
    P1 = np.cumsum(i * x64, 0)
    T0, T1 = P0[-1][None, :], P1[-1][None, :]
    Srow = (N * N - (i * (i + 1) / 2 + (N - 1 - i) * (N - i) / 2))
    Mx = (N * T0 - (2 * i * P0 - 2 * P1 + T1 - i * T0)) / Srow
    return ((1.0 - ALPHA) * (Mx.astype(np.float32) @ weight_time)).astype(np.float32)


def _build_nc():
    from concourse import bacc, tile, mybir
    from contextlib import ExitStack
    F32 = mybir.dt.float32
    F32R = mybir.dt.float32r
    BF16 = mybir.dt.bfloat16

    nc = bacc.Bacc()
    kt_d = nc.declare_dram_parameter("kt", [FEAT, N], F32R, isOutput=False)
    qt_d = nc.declare_dram_parameter("qt", [FEAT, NLOC], F32R, isOutput=False)
    xb_d = nc.declare_dram_parameter("xb", [N, IN], BF16, isOutput=False)
    o_ut = nc.declare_dram_parameter("o_ut", [IN, NLOC], F32, isOutput=True)
    o_z = nc.declare_dram_parameter("o_z", [P, NLOC], F32, isOutput=True)

    # k chunk column boundaries: fine 256-col chunks (2 key blocks each).
    # Tile deps are whole-tile, so each chunk is its own tile and S_b waits
    # only on the chunk containing its key block. Fine chunks let the DMA
    # schedule interleave k and x at per-block grain (queue rate ~210 GB/s
    # vs ~179 GB/s steady demand, so supply must never bunch up).
    KB = list(range(0, N + 1, 256))

    with tile.TileContext(nc) as tc, ExitStack() as ctx:
        cst = ctx.enter_context(tc.tile_pool(name="cst", bufs=1))
        xpool = ctx.enter_context(tc.tile_pool(name="xp", bufs=1))
        epool = ctx.enter_context(tc.tile_pool(name="ep", bufs=4))
        stg = ctx.enter_context(tc.tile_pool(name="stg", bufs=4))
        pss = ctx.enter_context(tc.tile_pool(name="pss", bufs=4, space="PSUM"))
        psu = ctx.enter_context(tc.tile_pool(name="psu", bufs=1, space="PSUM"))

        # separate q tile per m-half (whole-tile deps again)
        qtiles = [cst.tile([FEAT, HCOLS], F32R, name=f"q{h}")
                  for h in range(NLOC // HCOLS)]
        kchunks = [cst.tile([FEAT, KB[c + 1] - KB[c]], F32R, name=f"kc{c}")
                   for c in range(len(KB) - 1)]
        xtiles = [xpool.tile([P, IN], BF16, name=f"x{b}")
                  for b in range(NBLK)]

        def dma_k(c):
            nc.sync.dma_start(kchunks[c][:], kt_d[:, KB[c]:KB[c + 1]])

        def dma_x(b0, b1):
            for b in range(b0, b1):
                nc.sync.dma_start(xtiles[b][:], xb_d[b * P:(b + 1) * P, :])

        # supply-ordered DMA schedule at the measured ~210 GB/s queue rate:
        # q first, K0+K1 front-loaded (S_0..S_3 are prefetched before U_0
        # in PE order, so blocks 0-3 of k must beat x_0), then 1 k-chunk
        # per 2 x-blocks — k stays ~2 chunks ahead, x lands just-in-time
        nc.sync.dma_start(qtiles[0][:], qt_d[:, 0:HCOLS])
        dma_k(0)
        dma_k(1)
        dma_x(0, 1)
        dma_k(2)
        dma_x(1, 3)
        for c in range(3, len(KB) - 1):
            dma_k(c)
            dma_x(2 * c - 3, 2 * c - 1)
        dma_x(2 * (len(KB) - 1) - 3, NBLK)
        nc.sync.dma_start(qtiles[1][:], qt_d[:, HCOLS:NLOC])

        bias = cst.tile([P, 1], F32, name="bias")
        nc.vector.memset(bias[:], EXP_BIAS)
        zacc = cst.tile([P, NLOC], F32, name="zacc")
        nc.vector.memset(zacc[:], 0.0)

        # PE warm-up: ramp the tensor-engine clock while DMAs land
        wl = cst.tile([P, 64], BF16, name="wl")
        wr = cst.tile([P, P], BF16, name="wr")
        nc.vector.memset(wl[:], 0.0)
        nc.vector.memset(wr[:], 0.0)
        utiles = [psu.tile([P, HCOLS], F32, name=f"u{d}") for d in range(4)]
        for _ in range(8):
            nc.tensor.matmul(utiles[0][0:64, 0:P], wl[:], wr[:],
                             start=True, stop=True)

        for h in range(NLOC // HCOLS):
            hs = slice(h * HCOLS, (h + 1) * HCOLS)
            stash = {}

            def do_scores(b):
                sp = pss.tile([P, HCOLS], F32, tag="s")
                c, off = b // 2, (b % 2) * P
                nc.tensor.matmul(sp[:], kchunks[c][:, off:off + P],
                                 qtiles[h][:], start=True, stop=True)
                stash[b] = sp

            do_scores(0)
            do_scores(1)
            do_scores(2)
            for b in range(NBLK):
                if b + 3 < NBLK:
                    do_scores(b + 3)
                et = epool.tile([P, HCOLS], BF16, tag="e")
                nc.scalar.activation(et[:], stash.pop(b)[:],
                                     mybir.ActivationFunctionType.Exp,
                                     bias=bias[:])
                for d in range(4):
                    nc.tensor.matmul(utiles[d][:],
                                     xtiles[b][:, d * P:(d + 1) * P], et[:],
                                     start=(b == 0), stop=(b == NBLK - 1))
                nc.vector.tensor_tensor(zacc[:, hs], zacc[:, hs], et[:],
                                        mybir.AluOpType.add)

            nc.sync.dma_start(o_z[:, hs], zacc[:, hs])
            for d in range(4):
                st = stg.tile([P, HCOLS], F32, tag="st")
                if h == 1 and d % 2:
                    # tail half only: odd chunks copy via ACT (idle after its
                    # last exp) so the four drains run pairwise-parallel; for
                    # h=0 ACT copies would block h=1's exps in queue order
                    nc.scalar.activation(st[:], utiles[d][:],
                                         mybir.ActivationFunctionType.Copy)
                else:
                    nc.vector.tensor_copy(st[:], utiles[d][:])
                nc.sync.dma_start(o_ut[d * P:(d + 1) * P, hs], st[:])

    if not nc.is_finalized():
        nc.finalize()
    return nc


def _device_kernel(x, W0, W1, weight, weight_time, trace=False):
    sys.path.insert(0, "/opt/trn_rl_repo")
    import ml_dtypes
    from concourse.bass_utils import run_bass_kernel_spmd

    bf = ml_dtypes.bfloat16
    x = np.asarray(x, np.float32)
    W0 = np.asarray(W0, np.float32)
    W1 = np.asarray(W1, np.float32)
    weight = np.asarray(weight, np.float32)
    weight_time = np.asarray(weight_time, np.float32)

    q = x @ W0.T                       # [N, FEAT] fp32
    k = x @ W1.T
    kT = np.ascontiguousarray(k.T)     # [FEAT, N]
    qT = np.ascontiguousarray(q.T)
    xb = x.astype(bf)
    out_time = _toeplitz_out_time(x, weight_time)

    nc = _build_nc()
    in_maps = [dict(kt=kT, qt=np.ascontiguousarray(qT[:, c * NLOC:(c + 1) * NLOC]),
                    xb=xb) for c in range(NCORES)]

    kwargs = {}
    if trace:
        kwargs = dict(trace=True, trace_cores=list(range(NCORES)))
    res = run_bass_kernel_spmd(nc, in_maps, list(range(NCORES)), **kwargs)

    out = np.empty((N, NOUT), np.float32)
    for c in range(NCORES):
        r = res.results[c]
        sl = slice(c * NLOC, (c + 1) * NLOC)
        Z = r["o_z"].sum(0, dtype=np.float64).astype(np.float32)   # [NLOC]
        attn = (r["o_ut"].T @ weight) * (ALPHA / Z)[:, None]
        out[sl] = attn + out_time[sl]
    return out, res


def kernel(**inputs):
    try:
        out, _ = _device_kernel(**inputs)
        ref_dtype = np.asarray(inputs["x"]).dtype
        return out.astype(ref_dtype)
    except Exception:
        traceback.print_exc()
        sys.stderr.write("device path failed; using host fallback\n")
        return _host_reference(**inputs)
